# revision 74
# baseline (speedup 1.0000x reference)
"""Causal multi-head self-attention (b=4, s=2048, d_model=1024, 16 heads) on 8
Trainium2 NeuronCores.

Sharding: core c handles batch c//2 and head-group c%2 (8 of 16 heads):
wqkv row-split by head (tensor parallel), wo column-split; the host sums the
two partials of each batch while unsharding.

Design (v13, HW ~307-312 us; replaced a 327 us version. TensorMatrix busy
~260 us is the global binder; the ACT exp stream (~148 us) paces the
attention chunks, so projection/output groups are metered out as filler
wherever the PE would otherwise idle against it):
  - All inputs SBUF-resident. DMA queues deliver ~20-40 descriptors/us
    (1 descriptor per partition per dma_start), so initial loads are ordered
    first-needed-first across the 3 rings: sync carries the x chunks paced
    with A0's token chunks; scalar carries cos|sin (packed per 2-chunk
    halves) then wo; gpsimd carries wqk g0 split in Q/K halves (the first
    matmul's gate), wv, then late wqk groups. The mask/ident/perm/one-hot
    tables are BUILT ON-DEVICE (gpsimd memset + affine_select) -- no DMA.
    8 warm-up matmuls gated only on x0 ramp the PE out of its slow p-states
    before the first real group.
  - Projections are 8-matmul PSUM-accumulation groups (K=1024 via 8 eo
    slices, N=512): V token-major into vbuf (+ ones column via memset for
    free softmax denominators); Q/K feature-major with RoPE. Prologue (A0)
    uses the perm-matmul rope (cos-mult + P @ sin-mult, the permutation
    matmul deferred one group so the PE never waits on DVE); main-loop
    A-groups accumulate sin-products into a full row and swap-add via 4
    SWDGE descriptors on the idle gpsimd ring. A2/A3 are held back as
    hp1's/hp2's filler (without the holds the hooks drain every projection
    group during hp0 and later phases idle against the exp pace).
  - Attention per (head pair, q-chunk of 512) over causal k-tiles, software
    pipelined: 4 k-tiles of scores+exp lead their AV matmuls; the previous
    chunk's denominator tail plus ONE filler group are emitted right after
    the first lead k-tile, filling the exp-serialized lead phase and letting
    the DVE normalize finish before av(0) needs the single-buffered AV PSUM
    bank. Scores are a row-tiled matmul pair (heads at partition halves --
    they run concurrently in the two PE-array row-halves via tile_position),
    causal mask via ident@mtri accumulate, one exp per k-tile covering both
    heads' live columns, AV accumulates [V|1] so PSUM row 64 is the softmax
    denominator.
  - Denominator chain: DVE drains PSUM row 64 into rows 0/32 of a [33,512]
    tile (quadrant-aligned writes; filler rows pre-zeroed once), ONE K=33
    one-hot matmul broadcasts both heads' denominators across partition
    halves (512 PE rows), 1-pass DVE approx-reciprocal, normalize fused
    into the y^T PSUM drain.
  - Output projection pipelined into the last attention phase; both 512-col
    halves of a token tile drain into one [128,1024] bf16 tile and ship as
    ONE DMA round-robined over the 3 rings (descriptor pressure shapes the
    kernel tail); the final two tiles' DMAs are split 3 ways; the last
    chunk's normalize is split in 256-col pieces and tti 11 is held back so
    the trailing output groups overlap the final denominator chain.
"""

import sys

if "/opt/trn_rl_repo" not in sys.path:
    sys.path.insert(0, "/opt/trn_rl_repo")

from contextlib import ExitStack

import numpy as np

import concourse.bass as bass  # noqa: F401
import concourse.tile as tile
from concourse import bacc, mybir
from concourse.bass_utils import run_bass_kernel_spmd

F32 = mybir.dt.float32
F32R = mybir.dt.float32r
BF16 = mybir.dt.bfloat16
EXP = mybir.ActivationFunctionType.Exp
MULT = mybir.AluOpType.mult
ADD = mybir.AluOpType.add

B, S, D = 4, 2048, 1024
NH_CORE = 8      # heads per core
DH = 64          # head dim
P = 128
TCH = 512        # q/t chunk size
N_HP = NH_CORE // 2
NEG = -1.0e30
ROPE_THETA = 10000.0
SCALE = 1.0 / 8.0  # 1/sqrt(DH)

_CACHE = {}


def _emit(nc, tc, xTp, wqkp, wvp, wop, csp, outp):
    mm = nc.tensor.matmul
    n_kt = S // P  # 16

    with ExitStack() as ctx:
        # ---------------- persistent (resident) buffers ----------------
        persist = ctx.enter_context(tc.tile_pool(name="persist", bufs=1))
        # chunk-major so each load DMA writes contiguous 8KB/partition
        xT_sb = persist.tile([P, 4, 8, TCH], BF16, tag="xT", name="xT_sb")
        # head-pair-group major, half-major within a group: [g, half, ec, 128]
        # so group 0's Q-half (the first matmul's gate) loads on its own
        wqk_sb = persist.tile([P, 4, 2, 8, P], BF16, tag="wqk", name="wqk_sb")
        wv_sb = persist.tile([P, 8, 512], BF16, tag="wv", name="wv_sb")
        wo_sb = persist.tile([P, 4, 1024], BF16, tag="wo", name="wo_sb")
        # cos|sin packed chunk-major: one DMA per 512-token chunk loads both
        cs_sb = persist.tile([P, 4, 2, TCH], F32, tag="cs", name="cs_sb")
        qkT = [
            persist.tile([P, S], BF16, tag=f"qkT{ft}", name=f"qkT{ft}")
            for ft in range(8)
        ]
        vbuf = persist.tile([P, n_kt, NH_CORE, DH + 1], BF16, tag="vbuf", name="vbuf")
        yT = [
            persist.tile([P, S], BF16, tag=f"yT{hp}", name=f"yT{hp}")
            for hp in range(N_HP)
        ]
        # packed tables: mtri | ident | permP | onesh2, built on-device by
        # the idle GpSimd engine at t=0 (no DMA, no descriptors)
        const_sb = persist.tile([P, 4 * P], BF16, tag="const", name="const_sb")
        mtri_v = const_sb[:, 0:P]
        ident_v = const_sb[:, P : 2 * P]
        permP_v = const_sb[:, 2 * P : 3 * P]
        # one-hot rows 0 and 32 (engine writes are quadrant-aligned, so the
        # denominator copies land on partitions 0/32; rows 1..31 are zero)
        onesh2_v = const_sb[0:33, 3 * P : 4 * P]

        # ---------------- initial loads (3 queues, first-needed first) ----
        # Queues deliver ~20-40 descriptors/us (1 descriptor per partition
        # per dma_start), so completion time is queue-position-driven.
        # sync: x chunks paced with A0's token chunks; scalar: constp (the
        # deferred perm matmul needs it early), per-chunk cos|sin, wo;
        # gpsimd: wqk g0 by halves (the first matmul's gate), wv, wqk g1.
        nc.sync.dma_start(xT_sb[:, 0, :, :], xTp[0].ap()[:, :, :])
        nc.gpsimd.dma_start(wqk_sb[:, 0, 0, :, :], wqkp[0].ap()[:, 0, :, :])
        nc.scalar.dma_start(cs_sb[:, 0:2, :, :], csp.ap()[:, 0:2, :, :])
        nc.gpsimd.dma_start(wqk_sb[:, 0, 1, :, :], wqkp[0].ap()[:, 1, :, :])
        nc.sync.dma_start(xT_sb[:, 1, :, :], xTp[1].ap()[:, :, :])
        nc.scalar.dma_start(cs_sb[:, 2:4, :, :], csp.ap()[:, 2:4, :, :])
        nc.gpsimd.dma_start(wv_sb[:], wvp.ap()[:, :, :])
        nc.sync.dma_start(xT_sb[:, 2, :, :], xTp[2].ap()[:, :, :])
        nc.sync.dma_start(xT_sb[:, 3, :, :], xTp[3].ap()[:, :, :])
        nc.gpsimd.dma_start(wqk_sb[:, 1, :, :, :], wqkp[1].ap()[:, :, :, :])
        nc.scalar.dma_start(wo_sb[:], wop.ap()[:, :, :])
        nc.vector.memset(vbuf[:, :, :, DH : DH + 1], 1.0)

        # on-device table construction (GpSimd; after its dma issues so the
        # wqk descriptor generation isn't delayed)
        ISEQ = mybir.AluOpType.is_equal
        nc.gpsimd.memset(mtri_v, 0.0)
        nc.gpsimd.affine_select(
            out=mtri_v, in_=mtri_v, compare_op=mybir.AluOpType.is_ge,
            fill=NEG, base=0, pattern=[[1, P]], channel_multiplier=-1,
        )  # (f - p >= 0) ? 0 : NEG
        nc.gpsimd.memset(ident_v, 1.0)
        nc.gpsimd.affine_select(
            out=ident_v, in_=ident_v, compare_op=ISEQ,
            fill=0.0, base=0, pattern=[[-1, P]], channel_multiplier=1,
        )  # (p == f) ? 1 : 0
        nc.gpsimd.memset(permP_v, 1.0)
        permP4 = permP_v.rearrange("p (g h f) -> p g h f", g=2, h=2)
        nc.gpsimd.affine_select(
            out=permP4, in_=permP4, compare_op=ISEQ, fill=0.0, base=-32,
            pattern=[[-64, 2], [32, 2], [-1, 32]], channel_multiplier=1,
        )  # (p == 64g + 32 - 32h + f) ? 1 : 0  <=>  p == col ^ 32
        nc.gpsimd.memset(onesh2_v, 1.0)
        nc.gpsimd.affine_select(
            out=const_sb[0:33, 3 * P : 3 * P + 64],
            in_=const_sb[0:33, 3 * P : 3 * P + 64],
            compare_op=ISEQ, fill=0.0, base=0,
            pattern=[[0, 64]], channel_multiplier=1,
        )  # row 0 -> ones
        nc.gpsimd.affine_select(
            out=const_sb[0:33, 3 * P + 64 : 4 * P],
            in_=const_sb[0:33, 3 * P + 64 : 4 * P],
            compare_op=ISEQ, fill=0.0, base=-32,
            pattern=[[0, 64]], channel_multiplier=1,
        )  # row 32 -> ones

        # ---------------- SBUF working pools ----------------
        btpool = ctx.enter_context(tc.tile_pool(name="btmp", bufs=3))
        epool = ctx.enter_context(tc.tile_pool(name="expS", bufs=12))
        dpool = ctx.enter_context(tc.tile_pool(name="denst", bufs=2))
        rpool = ctx.enter_context(tc.tile_pool(name="recb", bufs=2))
        # zero the filler partitions 1..31 of both den buffers once; the
        # per-chunk copies only touch rows 0 and 32
        for _ in range(2):
            dz = dpool.tile([33, TCH], BF16, tag="den", name="den2")
            nc.vector.memset(dz[:], 0.0)

        # ---------------- projection group helpers ----------------
        def b_group(g, pj):
            # V projection for t-tile g (token-major), ones col via memset.
            flush_a()
            vps = pj.tile([P, TCH], F32, tag="pj", name="vps")
            tci, tt = divmod(g, 4)
            for ec in range(8):
                mm(
                    vps[:],
                    xT_sb[:, tci, ec, tt * P : (tt + 1) * P],
                    wv_sb[:, ec, :],
                    start=(ec == 0),
                    stop=(ec == 7),
                )
            nc.vector.tensor_copy(vbuf[:, g, :, 0:DH], vps[:])

        a_finish = []

        def flush_a():
            while a_finish:
                a_finish.pop(0)()

        bt_live = {}

        def a_group(ft, tci, pj, rope="dma"):
            # Q/K projection + rope for f-tile ft, token chunk tci.
            # rope="perm": qkT = cos*ps + P @ (sin_pm*ps) with the 32<->32
            #   row-swap done by one PE matmul (constant permutation lhsT);
            #   the perm matmul + add of the PREVIOUS group are emitted after
            #   this group's matmuls so the PE never waits a sin-mult in line.
            #   Used for the prologue (A0), where the SWDGE ring would pace
            #   the PE otherwise.
            # rope="dma": sin-products accumulate into a full-row btf and 4
            #   SWDGE swap-adds run per f-tile after the last chunk (the ring
            #   is idle mid-kernel; 4KB descriptors amortize best).
            tsl = slice(tci * TCH, (tci + 1) * TCH)
            cos_c = cs_sb[:, tci, 0, :]
            sin_c = cs_sb[:, tci, 1, :]
            g, half = ft % 4, ft // 4
            ps = pj.tile([P, TCH], F32, tag="pj", name="ps")
            for ec in range(8):
                mm(
                    ps[:],
                    wqk_sb[:, g, half, ec, :],
                    xT_sb[:, tci, ec, :],
                    start=(ec == 0),
                    stop=(ec == 7),
                )
            flush_a()
            if rope == "perm":
                btf = btpool.tile([P, TCH], BF16, tag="bt", name="btf")
                ctf = btpool.tile([P, TCH], BF16, tag="ct", name="ctf")
                nc.vector.tensor_tensor(btf[:], ps[:], sin_c, MULT)
                nc.vector.tensor_tensor(ctf[:], ps[:], cos_c, MULT)

                def finish():
                    ps2 = pj.tile([P, TCH], F32, tag="pj", name="ps2")
                    mm(ps2[:], permP_v, btf[:], start=True, stop=True)
                    nc.vector.tensor_tensor(qkT[ft][:, tsl], ctf[:], ps2[:], ADD)

                a_finish.append(finish)
            else:
                if tci == 0:
                    bt_live[ft] = btpool.tile([P, S], BF16, tag="btrow", name="btrow", bufs=2)
                btr = bt_live[ft]
                nc.vector.tensor_tensor(qkT[ft][:, tsl], ps[:], cos_c, MULT)
                nc.vector.tensor_tensor(btr[:, tsl], ps[:], sin_c, MULT)
                if tci == 3:
                    for blk in range(4):
                        a = blk * 32
                        c2 = a ^ 32  # partner half of the 64-row head block
                        nc.gpsimd.dma_start(
                            qkT[ft][c2 : c2 + 32, :],
                            btr[a : a + 32, :],
                            accum_op=ADD,
                        )

        # ---------------- attention chunk (software pipelined) ----------------
        LEAD = 4  # k-tiles of scores+exp emitted ahead of their AV matmuls

        def c_chunk(
            hp, qci, s_ps, av_ps, bc_ps, ktile_hook, prev_tail,
            fine_tail=False, lead_hook=None,
        ):
            # bc_ps: pool supplying the [128, 512] f32 PSUM tile for the
            # denominator-broadcast matmul (shared with proj/output pools).
            # Emits the first lead k-tile, then the previous chunk's
            # denominator tail (so its DVE normalize overlaps the remaining
            # lead scores and av(0) never waits on the single-buffered AV
            # PSUM), then the AV stream interleaved with the rest.
            qt = qkT[hp]
            ktt = qkT[4 + hp]
            h0, h1 = 2 * hp, 2 * hp + 1
            qsl = slice(qci * TCH, (qci + 1) * TCH)
            nkt = 4 * qci + 4
            avp = av_ps.tile([DH + 1, 2 * TCH], F32, tag="avp", name="avp")
            elive = {}

            def scores_exp(ki):
                ksl = slice(ki * P, (ki + 1) * P)
                diag = ki >= 4 * qci
                j = ki - 4 * qci
                off = j * P if diag else 0
                qlive = slice(qci * TCH + off, (qci + 1) * TCH)
                sp = s_ps.tile([P, 2 * TCH], F32, tag="sp", name="sp")
                mm(sp[:, off:TCH], ktt[0:64, ksl], qt[0:64, qlive], start=True, stop=True)
                mm(
                    sp[:, TCH + off : 2 * TCH],
                    ktt[64:128, ksl],
                    qt[64:128, qlive],
                    start=True,
                    stop=True,
                )
                if diag:
                    mm(
                        sp[:, off : off + P],
                        ident_v,
                        mtri_v,
                        start=False,
                        stop=True,
                        skip_group_check=True,
                    )
                    mm(
                        sp[:, TCH + off : TCH + off + P],
                        ident_v,
                        mtri_v,
                        start=False,
                        stop=True,
                        skip_group_check=True,
                    )
                e = epool.tile([P, 2 * TCH], BF16, tag="e", name="e")
                sp3 = sp[:].rearrange("p (h q) -> p h q", h=2)
                e3 = e[:].rearrange("p (h q) -> p h q", h=2)
                nc.scalar.activation(e3[:, :, off:], sp3[:, :, off:], EXP, scale=SCALE)
                elive[ki] = (e, off)

            def av(ki):
                e, off = elive.pop(ki)
                mm(
                    avp[:, off:TCH],
                    vbuf[:, ki, h0, :],
                    e[:, off:TCH],
                    start=(ki == 0),
                    stop=(ki == nkt - 1),
                    skip_group_check=True,
                )
                mm(
                    avp[:, TCH + off : 2 * TCH],
                    vbuf[:, ki, h1, :],
                    e[:, TCH + off : 2 * TCH],
                    start=(ki == 0),
                    stop=(ki == nkt - 1),
                    skip_group_check=True,
                )

            scores_exp(0)
            if prev_tail is not None:
                prev_tail()
            if lead_hook is not None:
                # one guaranteed filler group: the lead scores are serialized
                # to the exp stream, so the PE has ~2us of slack here
                lead_hook()
            for ki in range(1, min(LEAD, nkt)):
                scores_exp(ki)
            for ki in range(nkt):
                av(ki)
                if ki + LEAD < nkt:
                    scores_exp(ki + LEAD)
                ktile_hook()
            def tail():
                # denominator chain: DVE drains PSUM row 64 into rows 0/32
                # (h0/h1) -- on DVE (idle during chunk leads) so the copies
                # never interleave with the next chunk's exps on ACT, whose
                # pace the lead scores are serialized to.
                den2 = dpool.tile([33, TCH], BF16, tag="den", name="den2")
                nc.vector.tensor_copy(den2[0:1, :], avp[DH : DH + 1, 0:TCH])
                nc.vector.tensor_copy(den2[32:33, :], avp[DH : DH + 1, TCH : 2 * TCH])
                # ONE K=33 matmul broadcasts h0's denominators to partitions
                # 0:64 and h1's to 64:128 (block one-hot lhsT) -- 512 PE rows.
                rb = bc_ps.tile([P, TCH], F32, tag="pj", name="rb")
                mm(rb[:], onesh2_v, den2[:], start=True, stop=True, skip_group_check=True)
                rec = rpool.tile([P, TCH], F32, tag="rec", name="rec")
                nc.vector.reciprocal_approx_fast(out=rec[:], in_=rb[:])
                if not fine_tail:
                    nc.vector.tensor_tensor(
                        yT[hp][0:64, qsl], avp[0:DH, 0:TCH], rec[0:64, :], MULT
                    )
                    nc.vector.tensor_tensor(
                        yT[hp][64:128, qsl], avp[0:DH, TCH : 2 * TCH], rec[64:128, :], MULT
                    )
                else:
                    # 256-col pieces so trailing output-proj groups start
                    # after the first piece
                    for pc in range(2):
                        csl = slice(pc * 256, (pc + 1) * 256)
                        qpl = slice(qci * TCH + pc * 256, qci * TCH + (pc + 1) * 256)
                        nc.vector.tensor_tensor(
                            yT[hp][0:64, qpl], avp[0:DH, csl], rec[0:64, csl], MULT
                        )
                        nc.vector.tensor_tensor(
                            yT[hp][64:128, qpl],
                            avp[0:DH, TCH + pc * 256 : TCH + (pc + 1) * 256],
                            rec[64:128, csl],
                            MULT,
                        )

            return tail

        # ---------------- output projection group ----------------
        ot_live = {}

        def d_group(tti, jc, o_ps, osb, ring, drain="v", split_last=False):
            tsl = slice(tti * P, (tti + 1) * P)
            jsl = slice(jc * TCH, (jc + 1) * TCH)
            op = o_ps.tile([P, TCH], F32, tag="pj", name="op")
            for cc in range(4):
                mm(
                    op[:],
                    yT[cc][:, tsl],
                    wo_sb[:, cc, jsl],
                    start=(cc == 0),
                    stop=(cc == 3),
                )
            if jc == 0:
                ot_live[tti] = osb.tile([P, 2, TCH], BF16, tag="ot", name="ot")
            ot2 = ot_live[tti]
            if drain == "v":
                nc.vector.tensor_copy(ot2[:, jc, :], op[:])
            else:
                nc.scalar.copy(ot2[:, jc, :], op[:])
            if jc == 1:
                ot_live.pop(tti)
                if split_last:
                    t0 = tti * P
                    nc.sync.dma_start(outp.ap()[t0 : t0 + 48, :], ot2[0:48, :, :])
                    nc.scalar.dma_start(
                        outp.ap()[t0 + 48 : t0 + 96, :], ot2[48:96, :, :]
                    )
                    nc.gpsimd.dma_start(
                        outp.ap()[t0 + 96 : t0 + P, :], ot2[96:P, :, :]
                    )
                else:
                    ring.dma_start(outp.ap()[tsl, :], ot2[:, :, :])

        # ---------------- prologue: A0 only (it gates the first exp) ----
        with ExitStack() as pro:
            pj0 = pro.enter_context(tc.tile_pool(name="pj0", bufs=4, space="PSUM"))
            # p-state warm-up: matmuls gated only on the x0 DMA (which lands
            # ~3us before wqk g0) so the PE climbs out of its slow p-states
            # on throwaway work and the real groups start at full clock.
            wps = pj0.tile([P, TCH], F32, tag="pj", name="wps")
            for wi in range(8):
                mm(wps[:], xT_sb[:, 0, wi, 0:P], xT_sb[:, 0, wi, :],
                   start=(wi == 0), stop=(wi == 7))
            for tci in range(4):
                for ft in (0, 4):
                    a_group(ft, tci, pj0, rope="perm")
            for g in range(4):
                b_group(g, pj0)
            flush_a()
        nc.gpsimd.dma_start(wqk_sb[:, 2, :, :, :], wqkp[2].ap()[:, :, :, :])
        nc.gpsimd.dma_start(wqk_sb[:, 3, :, :, :], wqkp[3].ap()[:, :, :, :])

        # ---------------- main: C with interleaved proj groups ----------------
        with ExitStack() as cs:
            s_ps = cs.enter_context(tc.tile_pool(name="s_ps", bufs=2, space="PSUM"))
            av_ps = cs.enter_context(tc.tile_pool(name="av_ps", bufs=1, space="PSUM"))
            pj_stack = ExitStack()
            pjC = pj_stack.enter_context(
                tc.tile_pool(name="pjC", bufs=2, space="PSUM")
            )

            from collections import deque

            pendB = deque(range(4, 16))
            pendA = {
                h: deque((ft, tci) for ft in (h, 4 + h) for tci in range(4))
                for h in (1, 2, 3)
            }
            emittedB = [4]

            # A2/A3 are held back as hp1's/hp2's filler work: without the
            # holds the hooks drain every projection group during hp0 and
            # the later phases idle against the exp-paced attention stream
            # (phase map showed hp1 at 77-88% PE vs hp2's 90-100%).
            a_open = {1: True, 2: False, 3: False}

            def emit_next():
                # A-groups first: their full-row swap-adds must clear the
                # gpsimd ring before the next head pair's scores.
                for h in (1, 2, 3):
                    if not a_open[h]:
                        continue
                    if pendA[h]:
                        ft, tci = pendA[h].popleft()
                        a_group(ft, tci, pjC)
                        return True
                if pendB:
                    b_group(pendB.popleft(), pjC)
                    emittedB[0] += 1
                    return True
                return False

            def force_b(nkt):
                while emittedB[0] < nkt:
                    b_group(pendB.popleft(), pjC)
                    emittedB[0] += 1

            def force_a(h):
                while pendA[h]:
                    ft, tci = pendA[h].popleft()
                    a_group(ft, tci, pjC)
                flush_a()

            kglob = [0]

            def ktile_hook():
                kglob[0] += 1
                if kglob[0] % 2 == 0:
                    emit_next()

            tail = None
            for hp in range(3):
                if hp >= 1:
                    a_open[hp + 1] = True
                for h in range(1, hp + 1):
                    force_a(h)
                for qci in range(4):
                    force_b(4 * qci + 4)
                    tail = c_chunk(
                        hp, qci, s_ps, av_ps, pjC, ktile_hook, tail,
                        lead_hook=emit_next,
                    )

            # last head pair: free the proj bank, open output-proj PSUM
            force_a(3)
            tail()  # (2,3)'s denominator tail -- needs pjC, flush before close
            tail = None
            pj_stack.close()
            o_ps = cs.enter_context(tc.tile_pool(name="o_ps", bufs=2, space="PSUM"))
            osb = cs.enter_context(tc.tile_pool(name="osb", bufs=3))

            nohook = lambda: None  # noqa: E731
            d_next = [0]
            d_rings = [nc.sync, nc.scalar, nc.gpsimd]

            def emit_one_d():
                tti, jc = divmod(d_next[0], 2)
                d_group(tti, jc, o_ps, osb, d_rings[tti % 3])
                d_next[0] += 1

            def d_hook(max_tti):
                def h():
                    if d_next[0] < max_tti * 2:
                        emit_one_d()
                return h

            tail = c_chunk(3, 0, s_ps, av_ps, o_ps, nohook, tail)
            tail = c_chunk(3, 1, s_ps, av_ps, o_ps, d_hook(4), tail, lead_hook=d_hook(4))
            tail = c_chunk(3, 2, s_ps, av_ps, o_ps, d_hook(8), tail, lead_hook=d_hook(8))
            # hold tti 11 back: its two groups fill the PE bubble while the
            # final chunk's denominator chain runs
            tail = c_chunk(
                3, 3, s_ps, av_ps, o_ps, d_hook(11), tail,
                fine_tail=True, lead_hook=d_hook(11),
            )
            emit_one_d()
            emit_one_d()
            tail()

        # ---------------- tail of output projection ----------------
        with ExitStack() as ds:
            o2 = ds.enter_context(tc.tile_pool(name="o2", bufs=3, space="PSUM"))
            osb2 = ds.enter_context(tc.tile_pool(name="osb2", bufs=3))
            rings = [nc.sync, nc.scalar, nc.gpsimd]
            while d_next[0] < n_kt * 2:
                tti, jc = divmod(d_next[0], 2)
                d_group(
                    tti,
                    jc,
                    o2,
                    osb2,
                    rings[tti % 3],
                    drain="vs"[jc],
                    split_last=(tti >= n_kt - 2),
                )
                d_next[0] += 1


def _build():
    key = "nc_v3"
    if key in _CACHE:
        return _CACHE[key]
    nc = bacc.Bacc("TRN2", target_bir_lowering=False, debug=False, num_devices=8)
    xTp = [
        nc.dram_tensor(f"xTp{i}", [P, 8, TCH], BF16, kind="ExternalInput")
        for i in range(4)
    ]
    wqkp = [
        nc.dram_tensor(f"wqkp{g}", [P, 2, 8, P], BF16, kind="ExternalInput")
        for g in range(4)
    ]
    wvp = nc.dram_tensor("wvp", [P, 8, 512], BF16, kind="ExternalInput")
    wop = nc.dram_tensor("wop", [P, 4, 1024], BF16, kind="ExternalInput")
    csp = nc.dram_tensor("csp", [P, 4, 2, TCH], F32, kind="ExternalInput")
    outp = nc.dram_tensor("outp", [S, D], BF16, kind="ExternalOutput")
    with tile.TileContext(nc) as tc:
        _emit(nc, tc, xTp, wqkp, wvp, wop, csp, outp)
    nc.compile()
    _CACHE[key] = nc
    return nc


def host_inputs(x, wqkv, wo, token_positions):
    """Build the 8 per-core input maps (host-side sharding / layout prep)."""
    import ml_dtypes

    x = np.asarray(x, dtype=np.float32)
    wqkv = np.asarray(wqkv, dtype=np.float32)
    wo = np.asarray(wo, dtype=np.float32)
    pos = np.asarray(token_positions).astype(np.float32)

    d_model = x.shape[2]
    wq, wk, wv = wqkv[0:d_model], wqkv[d_model : 2 * d_model], wqkv[2 * d_model :]

    inv = np.float32(ROPE_THETA) ** (
        -np.arange(0, DH, 2, dtype=np.float32) / np.float32(DH)
    )  # [32]
    ang = pos[None, :] * inv[:, None]  # [32, S]
    cos32 = np.cos(ang).astype(np.float32)
    sin32 = np.sin(ang).astype(np.float32)
    cosp = np.tile(cos32, (4, 1))  # [128, S]
    sinp = np.tile(np.concatenate([sin32, -sin32], axis=0), (2, 1))  # [128, S]
    csp = np.empty((P, 4, 2, TCH), np.float32)
    for t in range(4):
        csp[:, t, 0, :] = cosp[:, t * TCH : (t + 1) * TCH]
        csp[:, t, 1, :] = sinp[:, t * TCH : (t + 1) * TCH]

    perm64 = np.concatenate([np.arange(0, DH, 2), np.arange(1, DH, 2)])

    def pmajor(mat, eo):
        # [eo*128, f] -> [128, eo, f]
        return np.ascontiguousarray(
            mat.reshape(eo, P, mat.shape[1]).transpose(1, 0, 2)
        ).astype(ml_dtypes.bfloat16)

    in_maps = []
    for ci in range(8):
        bi, hg = divmod(ci, 2)
        xT = np.ascontiguousarray(x[bi].T)  # [1024, 2048]
        xTr = pmajor(xT, 8)  # [128, 8, 2048]
        rows = []
        for blk in (wq, wk):
            for h in range(hg * NH_CORE, (hg + 1) * NH_CORE):
                rows.append(blk[h * DH : (h + 1) * DH][perm64])
        wqkT = np.ascontiguousarray(np.concatenate(rows, axis=0).T)  # [1024, 1024]
        wvT = np.ascontiguousarray(wv[hg * 512 : (hg + 1) * 512].T)  # [1024, 512]
        woT = np.ascontiguousarray(wo[:, hg * 512 : (hg + 1) * 512].T)  # [512, 1024]
        m = {
            "wvp": pmajor(wvT, 8),
            "wop": pmajor(woT, 4),
            "csp": csp,
        }
        for g in range(4):
            # [128, 2, 8, 128]: half-major (Q then K), ec-major within
            m[f"wqkp{g}"] = np.stack(
                [
                    pmajor(wqkT[:, g * P : (g + 1) * P], 8),
                    pmajor(wqkT[:, (4 + g) * P : (5 + g) * P], 8),
                ],
                axis=1,
            )
        for i in range(4):
            m[f"xTp{i}"] = np.ascontiguousarray(xTr[:, :, i * TCH : (i + 1) * TCH])
        in_maps.append(m)
    return in_maps


def _install_ntff_hook():
    """Recreate the antenv.axon_hooks NTFF profile hook this image lacks
    (same ctypes shim trn_agent_boot would register). Dev/profiling only."""
    import contextlib
    import ctypes
    import os
    import types

    try:
        import antenv.axon_hooks  # noqa: F401

        return
    except ImportError:
        pass
    so_path = "/opt/axon/libaxon_pjrt.so"
    if not os.path.exists(so_path):
        return
    lib = ctypes.CDLL(so_path)
    if not hasattr(lib, "axon_start_nrt_profile"):
        return
    lib.axon_start_nrt_profile.argtypes = [
        ctypes.POINTER(ctypes.c_int64),
        ctypes.c_size_t,
    ]
    lib.axon_start_nrt_profile.restype = ctypes.c_int64
    lib.axon_stop_nrt_profile.argtypes = [ctypes.c_char_p]
    lib.axon_stop_nrt_profile.restype = ctypes.c_int64

    @contextlib.contextmanager
    def _hook(output_dir, device_ids):
        import jax

        jax.devices()
        if device_ids:
            ids = (ctypes.c_int64 * len(device_ids))(*device_ids)
            rc = lib.axon_start_nrt_profile(ids, len(device_ids))
        else:
            rc = lib.axon_start_nrt_profile(None, 0)
        if rc != 0:
            raise RuntimeError(f"axon_start_nrt_profile rc={rc}")
        try:
            yield
        finally:
            n = lib.axon_stop_nrt_profile(str(output_dir).encode())
            if n < 0:
                raise RuntimeError(f"axon_stop_nrt_profile rc={n}")

    import antenv
    from concourse import bass_utils as _bu

    _bu.upload_artifacts = lambda d: d  # no bucket access in this container
    mod = types.ModuleType("antenv.axon_hooks")
    mod.get_axon_ntff_profile_hook = lambda: _hook
    mod.set_axon_ntff_profile_hook = lambda h: None
    sys.modules["antenv.axon_hooks"] = mod
    antenv.axon_hooks = mod


def kernel(x, wqkv, wo, token_positions, trace=False):
    if trace:
        _install_ntff_hook()
    nc = _build()
    in_maps = host_inputs(x, wqkv, wo, token_positions)
    res = run_bass_kernel_spmd(nc, in_maps, core_ids=list(range(8)), trace=trace)
    parts = [np.asarray(res.results[ci]["outp"]).astype(np.float32) for ci in range(8)]
    out = np.stack([parts[2 * bi] + parts[2 * bi + 1] for bi in range(B)], axis=0)
    if trace:
        kernel.last_result = res
    return out
